# revision 5
# baseline (speedup 1.0000x reference)
"""Mamba block (MockMambaBlock) on 8 Trainium2 NeuronCores.

Sharding: tensor-parallel over d_inner (8 x 256 channels), both batches on
every core. The x_proj/dt_proj contraction over d_inner is completed with an
on-device AllReduce of the small (B, 32, L) partial; out_proj row-partials
are summed on the host (the gather step).

Layout on device: channels on partitions, tokens along the free dimension,
with the SSM state index n laid out n-major along the free dim so the
per-(d,n) scan needs no cross-partition work. The d_state-broadcasts are
done with step-0 (broadcast) access patterns + a partition-replicating DMA.
"""

import sys

sys.path.insert(0, "/opt/trn_rl_repo")

import numpy as np
import ml_dtypes

import concourse.bass as bass
import concourse.bacc as bacc
import concourse.mybir as mybir
import concourse.tile as tile
from concourse.bass_utils import run_bass_kernel_spmd

F32 = mybir.dt.float32
F32R = mybir.dt.float32r
BF16 = mybir.dt.bfloat16
AF = mybir.ActivationFunctionType
OP = mybir.AluOpType

B, L, DM, DI, DS, DC = 2, 2048, 1024, 2048, 16, 4
NCORES = 8
DIL = DI // NCORES          # 256 channels per core
NBLK = DIL // 128           # 2 partition blocks of channels
KBLK = DM // 128            # 8 contraction blocks for in_proj
LTA = 512                   # phase A token chunk
LTB = 128                   # phase B token chunk
SCAN_DT = F32               # dtype for u/h scan tensors
Z_DT = BF16                 # dtype for silu(z) resident
XC_DT = BF16                # dtype for x_conv resident


def build_nc(ltb=LTB, scan_dt=SCAN_DT, z_dt=Z_DT, xc_dt=XC_DT):
    nc = bacc.Bacc()

    x_t = nc.dram_tensor("x_t", [B, KBLK, 128, L], F32R, kind="ExternalInput")
    win_d = nc.dram_tensor("win", [DM, 2 * DIL], F32R, kind="ExternalInput")
    wout_d = nc.dram_tensor("wout", [DIL, DM], F32R, kind="ExternalInput")
    wx_d = nc.dram_tensor("wx", [DIL, 2 * DS], BF16, kind="ExternalInput")
    wdt_d = nc.dram_tensor("wdt", [DS, DIL], F32R, kind="ExternalInput")
    a_d = nc.dram_tensor("a", [DIL, DS], F32, kind="ExternalInput")
    convw_d = nc.dram_tensor("convw", [DIL, DC], F32, kind="ExternalInput")
    convb_d = nc.dram_tensor("convb", [DIL, 1], F32, kind="ExternalInput")
    dvec_d = nc.dram_tensor("dvec", [DIL, 1], F32, kind="ExternalInput")
    bdt_d = nc.dram_tensor("bdt", [DIL, 1], F32, kind="ExternalInput")
    out_d = nc.dram_tensor("out_p", [B, L, DM], F32, kind="ExternalOutput")

    ncha = L // LTA
    nchb = L // ltb
    NMT = ltb // 128

    with tile.TileContext(nc) as tc:
        with (
            tc.tile_pool(name="weights", bufs=1) as wp,
            tc.tile_pool(name="resident", bufs=1) as rp,
            tc.tile_pool(name="dram", bufs=1, space="DRAM") as dp,
        ):
            # ---- weights to SBUF ----
            win_sb = wp.tile([128, KBLK, 2 * DIL], F32R)
            nc.sync.dma_start(win_sb[:], win_d[:].rearrange("(k p) m -> p k m", p=128))
            wout_sb = wp.tile([128, NBLK, DM], F32R)
            nc.sync.dma_start(wout_sb[:], wout_d[:].rearrange("(k p) m -> p k m", p=128))
            wx_sb = wp.tile([128, NBLK, 2 * DS], BF16)
            nc.sync.dma_start(wx_sb[:], wx_d[:].rearrange("(k p) m -> p k m", p=128))
            wdt_sb = wp.tile([DS, DIL], F32R)
            nc.sync.dma_start(wdt_sb[:], wdt_d[:])
            a_sb = wp.tile([128, NBLK, DS], F32)
            nc.sync.dma_start(a_sb[:], a_d[:].rearrange("(k p) m -> p k m", p=128))
            convw_sb = wp.tile([128, NBLK, DC], F32)
            nc.sync.dma_start(convw_sb[:], convw_d[:].rearrange("(k p) m -> p k m", p=128))
            convb_sb = wp.tile([128, NBLK, 1], F32)
            nc.sync.dma_start(convb_sb[:], convb_d[:].rearrange("(k p) m -> p k m", p=128))
            dvec_sb = wp.tile([128, NBLK, 1], F32)
            nc.sync.dma_start(dvec_sb[:], dvec_d[:].rearrange("(k p) m -> p k m", p=128))
            bdt_sb = wp.tile([128, NBLK, 1], F32)
            nc.sync.dma_start(bdt_sb[:], bdt_d[:].rearrange("(k p) m -> p k m", p=128))

            # ---- resident activations ----
            xcv = [[rp.tile([128, L], xc_dt, name=f"xcv{b_}{k}", tag=f"xcv{b_}{k}")
                    for k in range(NBLK)] for b_ in range(B)]
            zac = [[rp.tile([128, L], z_dt, name=f"zac{b_}{k}", tag=f"zac{b_}{k}")
                    for k in range(NBLK)] for b_ in range(B)]
            dtin_sb = [rp.tile([DS, L], F32R, name=f"dtin{b_}", tag=f"dtin{b_}")
                       for b_ in range(B)]
            md = [[rp.tile([128, L], BF16, name=f"md{b_}{k}", tag=f"md{b_}{k}")
                   for k in range(NBLK)] for b_ in range(B)]

            cc_in = dp.tile([B, 2 * DS, L], F32)
            cc_out = dp.tile([B, 2 * DS, L], F32, addr_space="Shared")

            # ================= Phase A =================
            with (
                tc.tile_pool(name="pa", bufs=2) as pa,
                tc.tile_pool(name="pa_ps", bufs=4, space="PSUM") as paps,
                tc.tile_pool(name="pa_ps1", bufs=1, space="PSUM") as paps1,
            ):
                xp_buf = [pa.tile([128, LTA + DC - 1], F32, name=f"xpb{k}",
                                  tag=f"xpb{k}", bufs=1) for k in range(NBLK)]
                for b_ in range(B):
                    ps_xs = paps1.tile([2 * DS, L], F32, tag="ps_xs")
                    for ch in range(ncha):
                        t0 = ch * LTA
                        xs_all = pa.tile([128, KBLK, LTA], F32R, tag="xs_all")
                        nc.sync.dma_start(
                            xs_all[:],
                            x_t[b_].transpose([1, 0, 2])[:, :, t0:t0 + LTA])
                        for m in range(2 * NBLK):
                            ps = paps.tile([128, LTA], F32, tag="ps_in")
                            for kb in range(KBLK):
                                nc.tensor.matmul(
                                    ps[:],
                                    win_sb[:, kb, m * 128:(m + 1) * 128],
                                    xs_all[:, kb, :],
                                    start=(kb == 0), stop=(kb == KBLK - 1))
                            if m < NBLK:  # x branch: conv + silu
                                blk = m
                                if ch == 0:
                                    nc.vector.memset(xp_buf[blk][:, 0:DC - 1], 0.0)
                                else:
                                    nc.vector.tensor_copy(
                                        xp_buf[blk][:, 0:DC - 1],
                                        xp_buf[blk][:, LTA:LTA + DC - 1])
                                nc.scalar.copy(xp_buf[blk][:, DC - 1:LTA + DC - 1], ps[:])
                                cacc = pa.tile([128, LTA], F32, tag="cacc")
                                nc.vector.tensor_scalar_mul(
                                    cacc[:], xp_buf[blk][:, 0:LTA],
                                    convw_sb[:, blk, 0:1])
                                for k in range(1, DC):
                                    nc.vector.scalar_tensor_tensor(
                                        cacc[:], xp_buf[blk][:, k:k + LTA],
                                        convw_sb[:, blk, k:k + 1], cacc[:],
                                        OP.mult, OP.add)
                                nc.scalar.activation(
                                    xcv[b_][blk][:, t0:t0 + LTA], cacc[:],
                                    AF.Silu, bias=convb_sb[:, blk, :])
                            else:  # z branch: silu
                                blk = m - NBLK
                                nc.scalar.activation(
                                    zac[b_][blk][:, t0:t0 + LTA], ps[:], AF.Silu)
                        # x_proj partial for this chunk
                        for kb in range(NBLK):
                            nc.tensor.matmul(
                                ps_xs[:, t0:t0 + LTA],
                                wx_sb[:, kb, :],
                                xcv[b_][kb][:, t0:t0 + LTA],
                                start=(kb == 0), stop=(kb == NBLK - 1))
                    xs_sb = pa.tile([2 * DS, L], F32, tag="xs_sb")
                    nc.scalar.copy(xs_sb[:], ps_xs[:])
                    nc.sync.dma_start(cc_in[b_], xs_sb[:])

            # ================= AllReduce =================
            nc.gpsimd.collective_compute(
                "AllReduce", OP.add,
                ins=[cc_in.opt()], outs=[cc_out.opt()],
                replica_groups=[list(range(NCORES))])
            for b_ in range(B):
                nc.sync.dma_start(dtin_sb[b_][:],
                                  cc_out[b_, 0:DS, :].bitcast(F32R))

            # ---- dt phase: md = -softplus(dt_raw + b_dt) = ln(sigmoid(-(dt_raw + b_dt)))
            LTD = 512
            with tc.tile_pool(name="pd_ps", bufs=4, space="PSUM") as pdps:
                for b_ in range(B):
                    for ch in range(L // LTD):
                        t0 = ch * LTD
                        for blk in range(NBLK):
                            ps_dt = pdps.tile([128, LTD], F32, tag="ps_dt")
                            nc.tensor.matmul(
                                ps_dt[:], wdt_sb[:, blk * 128:(blk + 1) * 128],
                                dtin_sb[b_][:, t0:t0 + LTD],
                                start=True, stop=True)
                            nc.scalar.activation(
                                md[b_][blk][:, t0:t0 + LTD], ps_dt[:],
                                AF.Sigmoid, bias=bdt_sb[:, blk, :], scale=-1.0)
                for b_ in range(B):
                    for blk in range(NBLK):
                        nc.scalar.activation(md[b_][blk][:], md[b_][blk][:], AF.Ln)

            # ================= Phase B =================
            with (
                tc.tile_pool(name="pb", bufs=2) as pb,
                tc.tile_pool(name="pb_ps", bufs=4, space="PSUM") as pbps,
            ):
                for b_ in range(B):
                    hstate = [pb.tile([128, DS], F32, name=f"hst{k}",
                                      tag=f"hst{k}", bufs=1) for k in range(NBLK)]
                    for ch in range(nchb):
                        t0 = ch * ltb
                        # B broadcast over partitions: Bb[p, n, t] = B_ssm[n, t]
                        bb = pb.tile([128, DS, ltb], F32, tag="bb", bufs=1)
                        nc.sync.dma_start(
                            bb[:],
                            cc_out[b_, DS:2 * DS, t0:t0 + ltb]
                            .rearrange("(o n) t -> o n t", o=1)
                            .broadcast_to([128, DS, ltb]))
                        for blk in range(NBLK):
                            dt_sb = md[b_][blk][:, t0:t0 + ltb]
                            # dtx = (-dt) * x_conv ; sign fixed by negated W_x B-cols
                            dtx = pb.tile([128, ltb], F32, tag=f"dtx{blk}")
                            nc.vector.tensor_mul(
                                dtx[:], dt_sb,
                                xcv[b_][blk][:, t0:t0 + ltb])
                            # dA = exp(A * dt), n-major along free
                            da = pb.tile([128, DS, ltb], F32, tag=f"da{blk}")
                            for n in range(DS):
                                nc.scalar.activation(
                                    da[:, n, :], dt_sb, AF.Exp,
                                    scale=a_sb[:, blk, n:n + 1])
                            # u = dtx * B  (broadcast dtx over n)
                            u = pb.tile([128, DS, ltb], scan_dt, tag=f"u{blk}")
                            nc.gpsimd.tensor_mul(
                                u[:],
                                dtx[:].rearrange("p (o t) -> p o t", o=1)
                                .broadcast_to([128, DS, ltb]),
                                bb[:])
                            # scan per n
                            h = pb.tile([128, DS, ltb], scan_dt, tag=f"h{blk}")
                            for n in range(DS):
                                init = 0.0 if ch == 0 else hstate[blk][:, n:n + 1]
                                nc.vector.tensor_tensor_scan(
                                    h[:, n, :], da[:, n, :], u[:, n, :],
                                    init, OP.mult, OP.add)
                            nc.vector.tensor_copy(hstate[blk][:],
                                                  h[:, :, ltb - 1])
                            # y = sum_n h
                            y = pb.tile([128, ltb], F32, tag=f"y{blk}")
                            nc.vector.tensor_reduce(
                                y[:], h[:].transpose([0, 2, 1]),
                                mybir.AxisListType.X, OP.add)
                            # gating: y_inner = (y + x_conv*D) * silu(z)
                            y1 = pb.tile([128, ltb], F32, tag=f"y1{blk}")
                            nc.vector.scalar_tensor_tensor(
                                y1[:], xcv[b_][blk][:, t0:t0 + ltb],
                                dvec_sb[:, blk, :], y[:], OP.mult, OP.add)
                            yin = pb.tile([128, ltb], F32R, tag=f"yin{blk}")
                            nc.vector.tensor_mul(yin[:], y1[:],
                                                 zac[b_][blk][:, t0:t0 + ltb])
                            if blk == 0:
                                yins = [yin]
                            else:
                                yins.append(yin)
                        # out_proj: out[t, dm] += yin.T @ wout
                        for mt in range(NMT):
                            for dmh in range(2):
                                ps_o = pbps.tile([128, 512], F32, tag="ps_o")
                                for blk in range(NBLK):
                                    nc.tensor.matmul(
                                        ps_o[:],
                                        yins[blk][:, mt * 128:(mt + 1) * 128],
                                        wout_sb[:, blk, dmh * 512:(dmh + 1) * 512],
                                        start=(blk == 0), stop=(blk == NBLK - 1))
                                osb = pb.tile([128, 512], F32, tag="osb")
                                nc.scalar.copy(osb[:], ps_o[:])
                                nc.sync.dma_start(
                                    out_d[b_, t0 + mt * 128:t0 + (mt + 1) * 128,
                                          dmh * 512:(dmh + 1) * 512],
                                    osb[:])

    nc.compile()
    return nc


_NC_CACHE = {}


def _get_nc():
    key = (LTB, SCAN_DT, Z_DT)
    if key not in _NC_CACHE:
        _NC_CACHE[key] = build_nc()
    return _NC_CACHE[key]


def make_in_maps(x, W_in, conv_w, conv_b, W_x, W_dt, b_dt, A_log, D, W_out):
    x = np.asarray(x, np.float32)
    W_in = np.asarray(W_in, np.float32)
    conv_w = np.asarray(conv_w, np.float32)
    conv_b = np.asarray(conv_b, np.float32)
    W_x = np.asarray(W_x, np.float32)
    W_dt = np.asarray(W_dt, np.float32)
    b_dt = np.asarray(b_dt, np.float32)
    A_log = np.asarray(A_log, np.float32)
    D = np.asarray(D, np.float32)
    W_out = np.asarray(W_out, np.float32)

    xt = np.ascontiguousarray(x.transpose(0, 2, 1)).reshape(B, KBLK, 128, L)
    A = np.exp(A_log)  # positive |A|; md = -dt on device

    in_maps = []
    for c in range(NCORES):
        lo = c * DIL
        sl = slice(lo, lo + DIL)
        in_maps.append({
            "x_t": xt,
            "win": np.ascontiguousarray(
                np.concatenate([W_in[:, sl], W_in[:, DI + lo:DI + lo + DIL]],
                               axis=1)),
            "wout": np.ascontiguousarray(W_out[sl]),
            "wx": np.ascontiguousarray(
                np.concatenate([W_x[sl, :DS], -W_x[sl, DS:]], axis=1)
            ).astype(ml_dtypes.bfloat16),
            "wdt": np.ascontiguousarray(W_dt[:, sl]),
            "a": np.ascontiguousarray(A[sl]),
            "convw": np.ascontiguousarray(conv_w[sl]),
            "convb": np.ascontiguousarray(conv_b[sl, None]),
            "dvec": np.ascontiguousarray(D[sl, None]),
            "bdt": np.ascontiguousarray(-b_dt[sl, None]),
        })
    return in_maps


def kernel(**inputs):
    nc = _get_nc()
    in_maps = make_in_maps(**inputs)
    res = run_bass_kernel_spmd(nc, in_maps, list(range(NCORES)))
    out = np.zeros((B, L, DM), np.float32)
    for c in range(NCORES):
        out += res.results[c]["out_p"]
    return out


# revision 7
# speedup vs baseline: 1.9809x; 1.9809x over previous
"""Mamba block (MockMambaBlock) on 8 Trainium2 NeuronCores.

Sharding: tensor-parallel over d_inner (8 x 256 channels), both batches on
every core. The x_proj/dt_proj contraction over d_inner is completed with an
on-device AllReduce of the small (B, 32, L) partial; out_proj row-partials
are summed on the host (the gather step).

Layout on device: channels on partitions, tokens along the free dimension,
with the SSM state index n laid out n-major along the free dim so the
per-(d,n) scan needs no cross-partition work. The d_state-broadcasts are
done with step-0 (broadcast) access patterns + a partition-replicating DMA.
"""

import sys

sys.path.insert(0, "/opt/trn_rl_repo")

import numpy as np
import ml_dtypes

import concourse.bass as bass
import concourse.bacc as bacc
import concourse.mybir as mybir
import concourse.tile as tile
from concourse.bass_utils import run_bass_kernel_spmd

F32 = mybir.dt.float32
F32R = mybir.dt.float32r
BF16 = mybir.dt.bfloat16
AF = mybir.ActivationFunctionType
OP = mybir.AluOpType

B, L, DM, DI, DS, DC = 2, 2048, 1024, 2048, 16, 4
NCORES = 8
DIL = DI // NCORES          # 256 channels per core
NBLK = DIL // 128           # 2 partition blocks of channels
KBLK = DM // 128            # 8 contraction blocks for in_proj
LTA = 512                   # phase A token chunk
LTB = 128                   # phase B token chunk
SCAN_DT = F32               # dtype for u/h scan tensors
Z_DT = BF16                 # dtype for silu(z) resident
XC_DT = BF16                # dtype for x_conv resident


def build_nc(ltb=LTB, scan_dt=SCAN_DT, z_dt=Z_DT, xc_dt=XC_DT):
    nc = bacc.Bacc()

    x_t = nc.dram_tensor("x_t", [B, KBLK, 128, L], F32R, kind="ExternalInput")
    win_d = nc.dram_tensor("win", [DM, 2 * DIL], F32R, kind="ExternalInput")
    wout_d = nc.dram_tensor("wout", [DIL, DM], F32R, kind="ExternalInput")
    wx_d = nc.dram_tensor("wx", [DIL, 2 * DS], BF16, kind="ExternalInput")
    wdt_d = nc.dram_tensor("wdt", [DS, DIL], F32R, kind="ExternalInput")
    a_d = nc.dram_tensor("a", [DIL, DS], F32, kind="ExternalInput")
    convw_d = nc.dram_tensor("convw", [DIL, DC], F32, kind="ExternalInput")
    convb_d = nc.dram_tensor("convb", [DIL, 1], F32, kind="ExternalInput")
    dvec_d = nc.dram_tensor("dvec", [DIL, 1], F32, kind="ExternalInput")
    bdt_d = nc.dram_tensor("bdt", [DIL, 1], F32, kind="ExternalInput")
    out_d = nc.dram_tensor("out_p", [B, L, DM], F32, kind="ExternalOutput")

    ncha = L // LTA
    nchb = L // ltb
    NMT = ltb // 128

    with tile.TileContext(nc) as tc:
        with (
            tc.tile_pool(name="weights", bufs=1) as wp,
            tc.tile_pool(name="resident", bufs=1) as rp,
            tc.tile_pool(name="dram", bufs=1, space="DRAM") as dp,
        ):
            # ---- weights to SBUF ----
            win_sb = wp.tile([128, KBLK, 2 * DIL], F32R)
            nc.sync.dma_start(win_sb[:], win_d[:].rearrange("(k p) m -> p k m", p=128))
            wout_sb = wp.tile([128, NBLK, DM], F32R)
            nc.sync.dma_start(wout_sb[:], wout_d[:].rearrange("(k p) m -> p k m", p=128))
            wx_sb = wp.tile([128, NBLK, 2 * DS], BF16)
            nc.sync.dma_start(wx_sb[:], wx_d[:].rearrange("(k p) m -> p k m", p=128))
            wdt_sb = wp.tile([DS, DIL], F32R)
            nc.sync.dma_start(wdt_sb[:], wdt_d[:])
            a_sb = wp.tile([128, NBLK, DS], F32)
            nc.sync.dma_start(a_sb[:], a_d[:].rearrange("(k p) m -> p k m", p=128))
            convw_sb = wp.tile([128, NBLK, DC], F32)
            nc.sync.dma_start(convw_sb[:], convw_d[:].rearrange("(k p) m -> p k m", p=128))
            convb_sb = wp.tile([128, NBLK, 1], F32)
            nc.sync.dma_start(convb_sb[:], convb_d[:].rearrange("(k p) m -> p k m", p=128))
            dvec_sb = wp.tile([128, NBLK, 1], F32)
            nc.sync.dma_start(dvec_sb[:], dvec_d[:].rearrange("(k p) m -> p k m", p=128))
            bdt_sb = wp.tile([128, NBLK, 1], F32)
            nc.sync.dma_start(bdt_sb[:], bdt_d[:].rearrange("(k p) m -> p k m", p=128))

            # ---- resident activations ----
            xcv = [[rp.tile([128, L], xc_dt, name=f"xcv{b_}{k}", tag=f"xcv{b_}{k}")
                    for k in range(NBLK)] for b_ in range(B)]
            zac = [[rp.tile([128, L], z_dt, name=f"zac{b_}{k}", tag=f"zac{b_}{k}")
                    for k in range(NBLK)] for b_ in range(B)]
            dtin_sb = [rp.tile([DS, L], F32R, name=f"dtin{b_}", tag=f"dtin{b_}")
                       for b_ in range(B)]
            md = [[rp.tile([128, L], BF16, name=f"md{b_}{k}", tag=f"md{b_}{k}")
                   for k in range(NBLK)] for b_ in range(B)]

            cc_in = dp.tile([B, 2 * DS, L], F32)
            cc_out = dp.tile([B, 2 * DS, L], F32, addr_space="Shared")

            # ================= Phase A =================
            with (
                tc.tile_pool(name="pa", bufs=2) as pa,
                tc.tile_pool(name="pa_ps", bufs=4, space="PSUM") as paps,
                tc.tile_pool(name="pa_ps1", bufs=1, space="PSUM") as paps1,
            ):
                xp_buf = [pa.tile([128, LTA + DC - 1], F32, name=f"xpb{k}",
                                  tag=f"xpb{k}", bufs=1) for k in range(NBLK)]
                for b_ in range(B):
                    ps_xs = paps1.tile([2 * DS, L], F32, tag="ps_xs")
                    for ch in range(ncha):
                        t0 = ch * LTA
                        xs_all = pa.tile([128, KBLK, LTA], F32R, tag="xs_all")
                        nc.sync.dma_start(
                            xs_all[:],
                            x_t[b_].transpose([1, 0, 2])[:, :, t0:t0 + LTA])
                        for m in range(2 * NBLK):
                            ps = paps.tile([128, LTA], F32, tag="ps_in")
                            for kb in range(KBLK):
                                nc.tensor.matmul(
                                    ps[:],
                                    win_sb[:, kb, m * 128:(m + 1) * 128],
                                    xs_all[:, kb, :],
                                    start=(kb == 0), stop=(kb == KBLK - 1))
                            if m < NBLK:  # x branch: conv + silu
                                blk = m
                                if ch == 0:
                                    nc.vector.memset(xp_buf[blk][:, 0:DC - 1], 0.0)
                                else:
                                    nc.vector.tensor_copy(
                                        xp_buf[blk][:, 0:DC - 1],
                                        xp_buf[blk][:, LTA:LTA + DC - 1])
                                nc.scalar.copy(xp_buf[blk][:, DC - 1:LTA + DC - 1], ps[:])
                                cacc = pa.tile([128, LTA], F32, tag="cacc")
                                nc.vector.tensor_scalar_mul(
                                    cacc[:], xp_buf[blk][:, 0:LTA],
                                    convw_sb[:, blk, 0:1])
                                for k in range(1, DC):
                                    nc.vector.scalar_tensor_tensor(
                                        cacc[:], xp_buf[blk][:, k:k + LTA],
                                        convw_sb[:, blk, k:k + 1], cacc[:],
                                        OP.mult, OP.add)
                                nc.scalar.activation(
                                    xcv[b_][blk][:, t0:t0 + LTA], cacc[:],
                                    AF.Silu, bias=convb_sb[:, blk, :])
                            else:  # z branch: silu
                                blk = m - NBLK
                                nc.scalar.activation(
                                    zac[b_][blk][:, t0:t0 + LTA], ps[:], AF.Silu)
                        # x_proj partial for this chunk
                        for kb in range(NBLK):
                            nc.tensor.matmul(
                                ps_xs[:, t0:t0 + LTA],
                                wx_sb[:, kb, :],
                                xcv[b_][kb][:, t0:t0 + LTA],
                                start=(kb == 0), stop=(kb == NBLK - 1))
                    xs_sb = pa.tile([2 * DS, L], F32, tag="xs_sb")
                    nc.scalar.copy(xs_sb[:], ps_xs[:])
                    nc.sync.dma_start(cc_in[b_], xs_sb[:])

            # ================= AllReduce =================
            nc.gpsimd.collective_compute(
                "AllReduce", OP.add,
                ins=[cc_in.opt()], outs=[cc_out.opt()],
                replica_groups=[list(range(NCORES))])
            for b_ in range(B):
                nc.sync.dma_start(dtin_sb[b_][:],
                                  cc_out[b_, 0:DS, :].bitcast(F32R))

            # ---- dt phase: md = -softplus(dt_raw + b_dt) = ln(sigmoid(-(dt_raw + b_dt)))
            LTD = 512
            with tc.tile_pool(name="pd_ps", bufs=4, space="PSUM") as pdps:
                for b_ in range(B):
                    for ch in range(L // LTD):
                        t0 = ch * LTD
                        for blk in range(NBLK):
                            ps_dt = pdps.tile([128, LTD], F32, tag="ps_dt")
                            nc.tensor.matmul(
                                ps_dt[:], wdt_sb[:, blk * 128:(blk + 1) * 128],
                                dtin_sb[b_][:, t0:t0 + LTD],
                                start=True, stop=True)
                            nc.scalar.activation(
                                md[b_][blk][:, t0:t0 + LTD], ps_dt[:],
                                AF.Sigmoid, bias=bdt_sb[:, blk, :], scale=-1.0)
                for b_ in range(B):
                    for blk in range(NBLK):
                        nc.scalar.activation(md[b_][blk][:], md[b_][blk][:], AF.Ln)

            # ================= Phase B =================
            with (
                tc.tile_pool(name="pb", bufs=2) as pb,
                tc.tile_pool(name="pb_ps", bufs=4, space="PSUM") as pbps,
            ):
                for b_ in range(B):
                    yins = {}
                    for blk in range(NBLK):
                        # dtx = (-dt) * x_conv (sign fixed via negated W_x B-cols)
                        dtx = pb.tile([128, L], F32, tag="dtx")
                        nc.vector.tensor_mul(dtx[:], md[b_][blk][:], xcv[b_][blk][:])
                        y = pb.tile([128, L], F32, tag="y")
                        for n in range(DS):
                            # B_ssm[n,:] replicated over partitions
                            bb = pb.tile([128, L], F32, tag="bbn", name=f"bb{b_}{blk}{n}")
                            nc.sync.dma_start(
                                bb[:],
                                cc_out[b_, DS + n:DS + n + 1, :].broadcast_to([128, L]))
                            # dA_n = exp(A[:, n] * md)
                            da = pb.tile([128, L], F32, tag="dan", name=f"da{b_}{blk}{n}")
                            nc.scalar.activation(da[:], md[b_][blk][:], AF.Exp,
                                                 scale=a_sb[:, blk, n:n + 1])
                            # u_n = dtx * B_n
                            u = pb.tile([128, L], F32, tag="un", name=f"u{b_}{blk}{n}")
                            nc.gpsimd.tensor_mul(u[:], dtx[:], bb[:])
                            # full-length scan, no chaining
                            h = pb.tile([128, L], BF16, tag="hn", name=f"h{b_}{blk}{n}")
                            nc.vector.tensor_tensor_scan(h[:], da[:], u[:],
                                                         0.0, OP.mult, OP.add)
                            if n == 0:
                                nc.vector.tensor_copy(y[:], h[:])
                            else:
                                nc.vector.tensor_add(y[:], y[:], h[:])
                        # gating: y_inner = (y + x_conv*D) * silu(z)
                        y1 = pb.tile([128, L], F32, tag="y1", bufs=1)
                        nc.vector.scalar_tensor_tensor(
                            y1[:], xcv[b_][blk][:], dvec_sb[:, blk, :], y[:],
                            OP.mult, OP.add)
                        yin = pb.tile([128, L], F32R, tag=f"yin{blk}", bufs=1)
                        nc.vector.tensor_mul(yin[:], y1[:], zac[b_][blk][:])
                        yins[blk] = yin
                    # out_proj: out[t, dm] += yin.T @ wout
                    for mt in range(L // 128):
                        for dmh in range(2):
                            ps_o = pbps.tile([128, 512], F32, tag="ps_o")
                            for blk in range(NBLK):
                                nc.tensor.matmul(
                                    ps_o[:],
                                    yins[blk][:, mt * 128:(mt + 1) * 128],
                                    wout_sb[:, blk, dmh * 512:(dmh + 1) * 512],
                                    start=(blk == 0), stop=(blk == NBLK - 1))
                            osb = pb.tile([128, 512], F32, tag="osb")
                            nc.scalar.copy(osb[:], ps_o[:])
                            nc.sync.dma_start(
                                out_d[b_, mt * 128:(mt + 1) * 128,
                                      dmh * 512:(dmh + 1) * 512],
                                osb[:])

    nc.compile()
    return nc


_NC_CACHE = {}


def _get_nc():
    key = (LTB, SCAN_DT, Z_DT)
    if key not in _NC_CACHE:
        _NC_CACHE[key] = build_nc()
    return _NC_CACHE[key]


def make_in_maps(x, W_in, conv_w, conv_b, W_x, W_dt, b_dt, A_log, D, W_out):
    x = np.asarray(x, np.float32)
    W_in = np.asarray(W_in, np.float32)
    conv_w = np.asarray(conv_w, np.float32)
    conv_b = np.asarray(conv_b, np.float32)
    W_x = np.asarray(W_x, np.float32)
    W_dt = np.asarray(W_dt, np.float32)
    b_dt = np.asarray(b_dt, np.float32)
    A_log = np.asarray(A_log, np.float32)
    D = np.asarray(D, np.float32)
    W_out = np.asarray(W_out, np.float32)

    xt = np.ascontiguousarray(x.transpose(0, 2, 1)).reshape(B, KBLK, 128, L)
    A = np.exp(A_log)  # positive |A|; md = -dt on device

    in_maps = []
    for c in range(NCORES):
        lo = c * DIL
        sl = slice(lo, lo + DIL)
        in_maps.append({
            "x_t": xt,
            "win": np.ascontiguousarray(
                np.concatenate([W_in[:, sl], W_in[:, DI + lo:DI + lo + DIL]],
                               axis=1)),
            "wout": np.ascontiguousarray(W_out[sl]),
            "wx": np.ascontiguousarray(
                np.concatenate([W_x[sl, :DS], -W_x[sl, DS:]], axis=1)
            ).astype(ml_dtypes.bfloat16),
            "wdt": np.ascontiguousarray(W_dt[:, sl]),
            "a": np.ascontiguousarray(A[sl]),
            "convw": np.ascontiguousarray(conv_w[sl]),
            "convb": np.ascontiguousarray(conv_b[sl, None]),
            "dvec": np.ascontiguousarray(D[sl, None]),
            "bdt": np.ascontiguousarray(-b_dt[sl, None]),
        })
    return in_maps


def kernel(**inputs):
    nc = _get_nc()
    in_maps = make_in_maps(**inputs)
    res = run_bass_kernel_spmd(nc, in_maps, list(range(NCORES)))
    out = np.zeros((B, L, DM), np.float32)
    for c in range(NCORES):
        out += res.results[c]["out_p"]
    return out


# revision 8
# speedup vs baseline: 2.4099x; 1.2166x over previous
"""Mamba block (MockMambaBlock) on 8 Trainium2 NeuronCores.

Sharding: tensor-parallel over d_inner (8 x 256 channels), both batches on
every core. The x_proj/dt_proj contraction over d_inner is completed with an
on-device AllReduce of the small (B, 32, L) partial; out_proj row-partials
are summed on the host (the gather step).

Layout on device: channels on partitions, tokens along the free dimension,
with the SSM state index n laid out n-major along the free dim so the
per-(d,n) scan needs no cross-partition work. The d_state-broadcasts are
done with step-0 (broadcast) access patterns + a partition-replicating DMA.
"""

import sys

sys.path.insert(0, "/opt/trn_rl_repo")

import numpy as np
import ml_dtypes

import concourse.bass as bass
import concourse.bacc as bacc
import concourse.mybir as mybir
import concourse.tile as tile
from concourse.bass_utils import run_bass_kernel_spmd

F32 = mybir.dt.float32
F32R = mybir.dt.float32r
BF16 = mybir.dt.bfloat16
AF = mybir.ActivationFunctionType
OP = mybir.AluOpType

B, L, DM, DI, DS, DC = 2, 2048, 1024, 2048, 16, 4
NCORES = 8
DIL = DI // NCORES          # 256 channels per core
NBLK = DIL // 128           # 2 partition blocks of channels
KBLK = DM // 128            # 8 contraction blocks for in_proj
LTA = 512                   # phase A token chunk
LTB = 128                   # phase B token chunk
SCAN_DT = F32               # dtype for u/h scan tensors
Z_DT = BF16                 # dtype for silu(z) resident
XC_DT = BF16                # dtype for x_conv resident


def build_nc(ltb=LTB, scan_dt=SCAN_DT, z_dt=Z_DT, xc_dt=XC_DT):
    nc = bacc.Bacc()

    x_t = nc.dram_tensor("x_t", [B, KBLK, 128, L], F32R, kind="ExternalInput")
    win_d = nc.dram_tensor("win", [DM, 2 * DIL], F32R, kind="ExternalInput")
    wout_d = nc.dram_tensor("wout", [DIL, DM], F32R, kind="ExternalInput")
    wx_d = nc.dram_tensor("wx", [DIL, 2 * DS], BF16, kind="ExternalInput")
    wdt_d = nc.dram_tensor("wdt", [DS, DIL], F32R, kind="ExternalInput")
    a_d = nc.dram_tensor("a", [DIL, DS], F32, kind="ExternalInput")
    convw_d = nc.dram_tensor("convw", [DIL, DC], F32, kind="ExternalInput")
    convb_d = nc.dram_tensor("convb", [DIL, 1], F32, kind="ExternalInput")
    dvec_d = nc.dram_tensor("dvec", [DIL, 1], F32, kind="ExternalInput")
    bdt_d = nc.dram_tensor("bdt", [DIL, 1], F32, kind="ExternalInput")
    identb_d = nc.dram_tensor("identb", [128, 128], BF16, kind="ExternalInput")
    diagd_d = nc.dram_tensor("diagd", [DIL, 128], BF16, kind="ExternalInput")
    out_d = nc.dram_tensor("out_p", [B, L, DM], F32, kind="ExternalOutput")

    ncha = L // LTA
    nchb = L // ltb
    NMT = ltb // 128

    with tile.TileContext(nc) as tc:
        with (
            tc.tile_pool(name="weights", bufs=1) as wp,
            tc.tile_pool(name="resident", bufs=1) as rp,
            tc.tile_pool(name="dram", bufs=1, space="DRAM") as dp,
        ):
            # ---- weights to SBUF ----
            win_sb = wp.tile([128, KBLK, 2 * DIL], F32R)
            nc.sync.dma_start(win_sb[:], win_d[:].rearrange("(k p) m -> p k m", p=128))
            wout_sb = wp.tile([128, NBLK, DM], F32R)
            nc.sync.dma_start(wout_sb[:], wout_d[:].rearrange("(k p) m -> p k m", p=128))
            wx_sb = wp.tile([128, NBLK, 2 * DS], BF16)
            nc.sync.dma_start(wx_sb[:], wx_d[:].rearrange("(k p) m -> p k m", p=128))
            wdt_sb = wp.tile([DS, DIL], F32R)
            nc.sync.dma_start(wdt_sb[:], wdt_d[:])
            a_sb = wp.tile([128, NBLK, DS], F32)
            nc.sync.dma_start(a_sb[:], a_d[:].rearrange("(k p) m -> p k m", p=128))
            convw_sb = wp.tile([128, NBLK, DC], F32)
            nc.sync.dma_start(convw_sb[:], convw_d[:].rearrange("(k p) m -> p k m", p=128))
            convb_sb = wp.tile([128, NBLK, 1], F32)
            nc.sync.dma_start(convb_sb[:], convb_d[:].rearrange("(k p) m -> p k m", p=128))
            dvec_sb = wp.tile([128, NBLK, 1], F32)
            nc.sync.dma_start(dvec_sb[:], dvec_d[:].rearrange("(k p) m -> p k m", p=128))
            bdt_sb = wp.tile([128, NBLK, 1], F32)
            nc.sync.dma_start(bdt_sb[:], bdt_d[:].rearrange("(k p) m -> p k m", p=128))
            identb_sb = wp.tile([128, 128], BF16)
            nc.sync.dma_start(identb_sb[:], identb_d[:])
            diagd_sb = wp.tile([128, NBLK, 128], BF16)
            nc.sync.dma_start(diagd_sb[:], diagd_d[:].rearrange("(k p) m -> p k m", p=128))

            # ---- resident activations ----
            xcv = [[rp.tile([128, L], xc_dt, name=f"xcv{b_}{k}", tag=f"xcv{b_}{k}")
                    for k in range(NBLK)] for b_ in range(B)]
            zac = [[rp.tile([128, L], z_dt, name=f"zac{b_}{k}", tag=f"zac{b_}{k}")
                    for k in range(NBLK)] for b_ in range(B)]
            dtin_sb = [rp.tile([DS, L], F32R, name=f"dtin{b_}", tag=f"dtin{b_}")
                       for b_ in range(B)]
            md = [[rp.tile([128, L], BF16, name=f"md{b_}{k}", tag=f"md{b_}{k}")
                   for k in range(NBLK)] for b_ in range(B)]

            cc_in = dp.tile([B, 2 * DS, L], F32)
            cc_out = dp.tile([B, 2 * DS, L], F32, addr_space="Shared")

            # ================= Phase A =================
            with (
                tc.tile_pool(name="pa", bufs=2) as pa,
                tc.tile_pool(name="pa_ps", bufs=4, space="PSUM") as paps,
                tc.tile_pool(name="pa_ps1", bufs=1, space="PSUM") as paps1,
            ):
                xp_buf = [pa.tile([128, LTA + DC - 1], F32, name=f"xpb{k}",
                                  tag=f"xpb{k}", bufs=1) for k in range(NBLK)]
                for b_ in range(B):
                    ps_xs = paps1.tile([2 * DS, L], F32, tag="ps_xs")
                    for ch in range(ncha):
                        t0 = ch * LTA
                        xs_all = pa.tile([128, KBLK, LTA], F32R, tag="xs_all")
                        nc.sync.dma_start(
                            xs_all[:],
                            x_t[b_].transpose([1, 0, 2])[:, :, t0:t0 + LTA])
                        for m in range(2 * NBLK):
                            ps = paps.tile([128, LTA], F32, tag="ps_in")
                            for kb in range(KBLK):
                                nc.tensor.matmul(
                                    ps[:],
                                    win_sb[:, kb, m * 128:(m + 1) * 128],
                                    xs_all[:, kb, :],
                                    start=(kb == 0), stop=(kb == KBLK - 1))
                            if m < NBLK:  # x branch: conv + silu
                                blk = m
                                if ch == 0:
                                    nc.vector.memset(xp_buf[blk][:, 0:DC - 1], 0.0)
                                else:
                                    nc.vector.tensor_copy(
                                        xp_buf[blk][:, 0:DC - 1],
                                        xp_buf[blk][:, LTA:LTA + DC - 1])
                                nc.scalar.copy(xp_buf[blk][:, DC - 1:LTA + DC - 1], ps[:])
                                cacc = pa.tile([128, LTA], F32, tag="cacc")
                                nc.vector.tensor_scalar_mul(
                                    cacc[:], xp_buf[blk][:, 0:LTA],
                                    convw_sb[:, blk, 0:1])
                                for k in range(1, DC):
                                    nc.vector.scalar_tensor_tensor(
                                        cacc[:], xp_buf[blk][:, k:k + LTA],
                                        convw_sb[:, blk, k:k + 1], cacc[:],
                                        OP.mult, OP.add)
                                nc.scalar.activation(
                                    xcv[b_][blk][:, t0:t0 + LTA], cacc[:],
                                    AF.Silu, bias=convb_sb[:, blk, :])
                            else:  # z branch: silu
                                blk = m - NBLK
                                nc.scalar.activation(
                                    zac[b_][blk][:, t0:t0 + LTA], ps[:], AF.Silu)
                        # x_proj partial for this chunk
                        for kb in range(NBLK):
                            nc.tensor.matmul(
                                ps_xs[:, t0:t0 + LTA],
                                wx_sb[:, kb, :],
                                xcv[b_][kb][:, t0:t0 + LTA],
                                start=(kb == 0), stop=(kb == NBLK - 1))
                    xs_sb = pa.tile([2 * DS, L], F32, tag="xs_sb")
                    nc.scalar.copy(xs_sb[:], ps_xs[:])
                    nc.sync.dma_start(cc_in[b_], xs_sb[:])

            # ================= AllReduce =================
            nc.gpsimd.collective_compute(
                "AllReduce", OP.add,
                ins=[cc_in.opt()], outs=[cc_out.opt()],
                replica_groups=[list(range(NCORES))])
            for b_ in range(B):
                nc.sync.dma_start(dtin_sb[b_][:],
                                  cc_out[b_, 0:DS, :].bitcast(F32R))

            # ---- dt phase: md = -softplus(dt_raw + b_dt) = ln(sigmoid(-(dt_raw + b_dt)))
            LTD = 512
            with tc.tile_pool(name="pd_ps", bufs=4, space="PSUM") as pdps:
                for b_ in range(B):
                    for ch in range(L // LTD):
                        t0 = ch * LTD
                        for blk in range(NBLK):
                            ps_dt = pdps.tile([128, LTD], F32, tag="ps_dt")
                            nc.tensor.matmul(
                                ps_dt[:], wdt_sb[:, blk * 128:(blk + 1) * 128],
                                dtin_sb[b_][:, t0:t0 + LTD],
                                start=True, stop=True)
                            nc.scalar.activation(
                                md[b_][blk][:, t0:t0 + LTD], ps_dt[:],
                                AF.Sigmoid, bias=bdt_sb[:, blk, :], scale=-1.0)
                for b_ in range(B):
                    for blk in range(NBLK):
                        nc.scalar.activation(md[b_][blk][:], md[b_][blk][:], AF.Ln)

            # ================= Phase B =================
            with (
                tc.tile_pool(name="pb", bufs=2) as pb,
                tc.tile_pool(name="pb_ps", bufs=4, space="PSUM") as pbps,
            ):
                NPT = L // 512
                for b_ in range(B):
                    yins = {}
                    for blk in range(NBLK):
                        # dtx = (-dt) * x_conv (sign fixed via negated W_x B-cols)
                        dtx = pb.tile([128, L], F32, tag="dtx")
                        nc.vector.tensor_mul(dtx[:], md[b_][blk][:], xcv[b_][blk][:])
                        y_ps = [pbps.tile([128, 512], F32, tag="y_ps", bufs=4,
                                          name=f"yps{b_}{blk}{pt}")
                                for pt in range(NPT)]
                        for n in range(DS):
                            # B_ssm[n,:] replicated over partitions
                            bb = pb.tile([128, L], F32, tag="bbn", name=f"bb{b_}{blk}{n}")
                            nc.sync.dma_start(
                                bb[:],
                                cc_out[b_, DS + n:DS + n + 1, :].broadcast_to([128, L]))
                            # dA_n = exp(A[:, n] * md)
                            da = pb.tile([128, L], F32, tag="dan", name=f"da{b_}{blk}{n}")
                            nc.scalar.activation(da[:], md[b_][blk][:], AF.Exp,
                                                 scale=a_sb[:, blk, n:n + 1])
                            # u_n = dtx * B_n
                            u = pb.tile([128, L], F32, tag="un", name=f"u{b_}{blk}{n}")
                            eng = nc.vector if n >= 13 else nc.gpsimd
                            eng.tensor_mul(u[:], dtx[:], bb[:])
                            # full-length scan, no chaining
                            h = pb.tile([128, L], BF16, tag="hn", name=f"h{b_}{blk}{n}")
                            nc.vector.tensor_tensor_scan(h[:], da[:], u[:],
                                                         0.0, OP.mult, OP.add)
                            # y += h_n on the tensor engine (identity matmul)
                            for pt in range(NPT):
                                nc.tensor.matmul(
                                    y_ps[pt][:], identb_sb[:],
                                    h[:, pt * 512:(pt + 1) * 512],
                                    start=(n == 0), stop=False)
                        # y += x_conv * D via diag(D) matmul, then gate from PSUM
                        yin = pb.tile([128, L], F32R, tag=f"yin{blk}", bufs=1)
                        for pt in range(NPT):
                            nc.tensor.matmul(
                                y_ps[pt][:], diagd_sb[:, blk, :],
                                xcv[b_][blk][:, pt * 512:(pt + 1) * 512],
                                start=False, stop=True)
                            nc.vector.tensor_mul(
                                yin[:, pt * 512:(pt + 1) * 512], y_ps[pt][:],
                                zac[b_][blk][:, pt * 512:(pt + 1) * 512])
                        yins[blk] = yin
                    # out_proj: out[t, dm] += yin.T @ wout
                    for mt in range(L // 128):
                        for dmh in range(2):
                            ps_o = pbps.tile([128, 512], F32, tag="ps_o")
                            for blk in range(NBLK):
                                nc.tensor.matmul(
                                    ps_o[:],
                                    yins[blk][:, mt * 128:(mt + 1) * 128],
                                    wout_sb[:, blk, dmh * 512:(dmh + 1) * 512],
                                    start=(blk == 0), stop=(blk == NBLK - 1))
                            osb = pb.tile([128, 512], F32, tag="osb")
                            nc.scalar.copy(osb[:], ps_o[:])
                            nc.sync.dma_start(
                                out_d[b_, mt * 128:(mt + 1) * 128,
                                      dmh * 512:(dmh + 1) * 512],
                                osb[:])

    nc.compile()
    return nc


_NC_CACHE = {}


def _get_nc():
    key = (LTB, SCAN_DT, Z_DT)
    if key not in _NC_CACHE:
        _NC_CACHE[key] = build_nc()
    return _NC_CACHE[key]


def make_in_maps(x, W_in, conv_w, conv_b, W_x, W_dt, b_dt, A_log, D, W_out):
    x = np.asarray(x, np.float32)
    W_in = np.asarray(W_in, np.float32)
    conv_w = np.asarray(conv_w, np.float32)
    conv_b = np.asarray(conv_b, np.float32)
    W_x = np.asarray(W_x, np.float32)
    W_dt = np.asarray(W_dt, np.float32)
    b_dt = np.asarray(b_dt, np.float32)
    A_log = np.asarray(A_log, np.float32)
    D = np.asarray(D, np.float32)
    W_out = np.asarray(W_out, np.float32)

    xt = np.ascontiguousarray(x.transpose(0, 2, 1)).reshape(B, KBLK, 128, L)
    A = np.exp(A_log)  # positive |A|; md = -dt on device

    in_maps = []
    for c in range(NCORES):
        lo = c * DIL
        sl = slice(lo, lo + DIL)
        in_maps.append({
            "x_t": xt,
            "win": np.ascontiguousarray(
                np.concatenate([W_in[:, sl], W_in[:, DI + lo:DI + lo + DIL]],
                               axis=1)),
            "wout": np.ascontiguousarray(W_out[sl]),
            "wx": np.ascontiguousarray(
                np.concatenate([W_x[sl, :DS], -W_x[sl, DS:]], axis=1)
            ).astype(ml_dtypes.bfloat16),
            "wdt": np.ascontiguousarray(W_dt[:, sl]),
            "a": np.ascontiguousarray(A[sl]),
            "convw": np.ascontiguousarray(conv_w[sl]),
            "convb": np.ascontiguousarray(conv_b[sl, None]),
            "dvec": np.ascontiguousarray(D[sl, None]),
            "bdt": np.ascontiguousarray(-b_dt[sl, None]),
            "identb": np.eye(128, dtype=ml_dtypes.bfloat16),
            "diagd": np.stack([np.diag(D[lo + k * 128:lo + (k + 1) * 128])
                               for k in range(NBLK)]).reshape(DIL, 128)
                       .astype(ml_dtypes.bfloat16),
        })
    return in_maps


def kernel(**inputs):
    nc = _get_nc()
    in_maps = make_in_maps(**inputs)
    res = run_bass_kernel_spmd(nc, in_maps, list(range(NCORES)))
    out = np.zeros((B, L, DM), np.float32)
    for c in range(NCORES):
        out += res.results[c]["out_p"]
    return out


# revision 10
# speedup vs baseline: 2.5450x; 1.0561x over previous
"""Mamba block (MockMambaBlock) on 8 Trainium2 NeuronCores.

Sharding: tensor-parallel over d_inner (8 x 256 channels), both batches on
every core. The x_proj/dt_proj contraction over d_inner is completed with an
on-device AllReduce of the small (B, 32, L) partial; out_proj row-partials
are summed on the host (the gather step).

Layout on device: channels on partitions, tokens along the free dimension,
with the SSM state index n laid out n-major along the free dim so the
per-(d,n) scan needs no cross-partition work. The d_state-broadcasts are
done with step-0 (broadcast) access patterns + a partition-replicating DMA.
"""

import sys

sys.path.insert(0, "/opt/trn_rl_repo")

import numpy as np
import ml_dtypes

import concourse.bass as bass
import concourse.bacc as bacc
import concourse.mybir as mybir
import concourse.tile as tile
from concourse.bass_utils import run_bass_kernel_spmd

F32 = mybir.dt.float32
F32R = mybir.dt.float32r
BF16 = mybir.dt.bfloat16
AF = mybir.ActivationFunctionType
OP = mybir.AluOpType

B, L, DM, DI, DS, DC = 2, 2048, 1024, 2048, 16, 4
NCORES = 8
DIL = DI // NCORES          # 256 channels per core
NBLK = DIL // 128           # 2 partition blocks of channels
KBLK = DM // 128            # 8 contraction blocks for in_proj
LTA = 512                   # phase A token chunk
LTB = 128                   # phase B token chunk
SCAN_DT = F32               # dtype for u/h scan tensors
Z_DT = BF16                 # dtype for silu(z) resident
XC_DT = BF16                # dtype for x_conv resident


def build_nc(ltb=LTB, scan_dt=SCAN_DT, z_dt=Z_DT, xc_dt=XC_DT):
    nc = bacc.Bacc()

    x_t = nc.dram_tensor("x_t", [B, KBLK, 128, L], F32R, kind="ExternalInput")
    win_d = nc.dram_tensor("win", [DM, 2 * DIL], F32R, kind="ExternalInput")
    wout_d = nc.dram_tensor("wout", [DIL, DM], F32R, kind="ExternalInput")
    wx_d = nc.dram_tensor("wx", [DIL, 2 * DS], BF16, kind="ExternalInput")
    wdt_d = nc.dram_tensor("wdt", [DS, DIL], F32R, kind="ExternalInput")
    a_d = nc.dram_tensor("a", [DIL, DS], F32, kind="ExternalInput")
    convw_d = nc.dram_tensor("convw", [DIL, DC], F32, kind="ExternalInput")
    convb_d = nc.dram_tensor("convb", [DIL, 1], F32, kind="ExternalInput")
    dvec_d = nc.dram_tensor("dvec", [DIL, 1], F32, kind="ExternalInput")
    bdt_d = nc.dram_tensor("bdt", [DIL, 1], F32, kind="ExternalInput")
    identb_d = nc.dram_tensor("identb", [128, 128], BF16, kind="ExternalInput")
    diagd_d = nc.dram_tensor("diagd", [DIL, 128], BF16, kind="ExternalInput")
    out_d = nc.dram_tensor("out_p", [B, L, DM], F32, kind="ExternalOutput")

    ncha = L // LTA
    nchb = L // ltb
    NMT = ltb // 128

    with tile.TileContext(nc) as tc:
        with (
            tc.tile_pool(name="weights", bufs=1) as wp,
            tc.tile_pool(name="resident", bufs=1) as rp,
            tc.tile_pool(name="dram", bufs=1, space="DRAM") as dp,
        ):
            # ---- weights to SBUF ----
            win_sb = wp.tile([128, KBLK, 2 * DIL], F32R)
            nc.sync.dma_start(win_sb[:], win_d[:].rearrange("(k p) m -> p k m", p=128))
            wout_sb = wp.tile([128, NBLK, DM], F32R)
            nc.sync.dma_start(wout_sb[:], wout_d[:].rearrange("(k p) m -> p k m", p=128))
            wx_sb = wp.tile([128, NBLK, 2 * DS], BF16)
            nc.sync.dma_start(wx_sb[:], wx_d[:].rearrange("(k p) m -> p k m", p=128))
            wdt_sb = wp.tile([DS, DIL], F32R)
            nc.sync.dma_start(wdt_sb[:], wdt_d[:])
            a_sb = wp.tile([128, NBLK, DS], F32)
            nc.sync.dma_start(a_sb[:], a_d[:].rearrange("(k p) m -> p k m", p=128))
            convw_sb = wp.tile([128, NBLK, DC], F32)
            nc.sync.dma_start(convw_sb[:], convw_d[:].rearrange("(k p) m -> p k m", p=128))
            convb_sb = wp.tile([128, NBLK, 1], F32)
            nc.sync.dma_start(convb_sb[:], convb_d[:].rearrange("(k p) m -> p k m", p=128))
            dvec_sb = wp.tile([128, NBLK, 1], F32)
            nc.sync.dma_start(dvec_sb[:], dvec_d[:].rearrange("(k p) m -> p k m", p=128))
            bdt_sb = wp.tile([128, NBLK, 1], F32)
            nc.sync.dma_start(bdt_sb[:], bdt_d[:].rearrange("(k p) m -> p k m", p=128))
            identb_sb = wp.tile([128, 128], BF16)
            nc.sync.dma_start(identb_sb[:], identb_d[:])
            diagd_sb = wp.tile([128, NBLK, 128], BF16)
            nc.sync.dma_start(diagd_sb[:], diagd_d[:].rearrange("(k p) m -> p k m", p=128))

            # ---- resident activations ----
            xcv = [[rp.tile([128, L], xc_dt, name=f"xcv{b_}{k}", tag=f"xcv{b_}{k}")
                    for k in range(NBLK)] for b_ in range(B)]
            zac = [[rp.tile([128, L], z_dt, name=f"zac{b_}{k}", tag=f"zac{b_}{k}")
                    for k in range(NBLK)] for b_ in range(B)]
            dtin_sb = [rp.tile([DS, L], F32R, name=f"dtin{b_}", tag=f"dtin{b_}")
                       for b_ in range(B)]
            md = [[rp.tile([128, L], BF16, name=f"md{b_}{k}", tag=f"md{b_}{k}")
                   for k in range(NBLK)] for b_ in range(B)]

            cc_in = [dp.tile([2 * DS, L], F32, name=f"cc_in{b_}") for b_ in range(B)]
            cc_out = [dp.tile([2 * DS, L], F32, addr_space="Shared",
                              name=f"cc_out{b_}") for b_ in range(B)]

            # ================= Phase A =================
            with (
                tc.tile_pool(name="pa", bufs=2) as pa,
                tc.tile_pool(name="pa_ps", bufs=4, space="PSUM") as paps,
                tc.tile_pool(name="pa_ps1", bufs=1, space="PSUM") as paps1,
            ):
                xp_buf = [pa.tile([128, LTA + DC - 1], F32, name=f"xpb{k}",
                                  tag=f"xpb{k}", bufs=1) for k in range(NBLK)]
                for b_ in range(B):
                    ps_xs = paps1.tile([2 * DS, L], F32, tag="ps_xs")
                    for ch in range(ncha):
                        t0 = ch * LTA
                        xs_all = pa.tile([128, KBLK, LTA], F32R, tag="xs_all")
                        nc.sync.dma_start(
                            xs_all[:],
                            x_t[b_].transpose([1, 0, 2])[:, :, t0:t0 + LTA])
                        for m in range(2 * NBLK):
                            ps = paps.tile([128, LTA], F32, tag="ps_in")
                            for kb in range(KBLK):
                                nc.tensor.matmul(
                                    ps[:],
                                    win_sb[:, kb, m * 128:(m + 1) * 128],
                                    xs_all[:, kb, :],
                                    start=(kb == 0), stop=(kb == KBLK - 1))
                            if m < NBLK:  # x branch: conv + silu
                                blk = m
                                if ch == 0:
                                    nc.vector.memset(xp_buf[blk][:, 0:DC - 1], 0.0)
                                else:
                                    nc.vector.tensor_copy(
                                        xp_buf[blk][:, 0:DC - 1],
                                        xp_buf[blk][:, LTA:LTA + DC - 1])
                                nc.scalar.copy(xp_buf[blk][:, DC - 1:LTA + DC - 1], ps[:])
                                cacc = pa.tile([128, LTA], F32, tag="cacc")
                                nc.vector.tensor_scalar_mul(
                                    cacc[:], xp_buf[blk][:, 0:LTA],
                                    convw_sb[:, blk, 0:1])
                                for k in range(1, DC):
                                    nc.vector.scalar_tensor_tensor(
                                        cacc[:], xp_buf[blk][:, k:k + LTA],
                                        convw_sb[:, blk, k:k + 1], cacc[:],
                                        OP.mult, OP.add)
                                nc.scalar.activation(
                                    xcv[b_][blk][:, t0:t0 + LTA], cacc[:],
                                    AF.Silu, bias=convb_sb[:, blk, :])
                            else:  # z branch: silu
                                blk = m - NBLK
                                nc.scalar.activation(
                                    zac[b_][blk][:, t0:t0 + LTA], ps[:], AF.Silu)
                        # x_proj partial for this chunk
                        for kb in range(NBLK):
                            nc.tensor.matmul(
                                ps_xs[:, t0:t0 + LTA],
                                wx_sb[:, kb, :],
                                xcv[b_][kb][:, t0:t0 + LTA],
                                start=(kb == 0), stop=(kb == NBLK - 1))
                    xs_sb = pa.tile([2 * DS, L], F32, tag="xs_sb")
                    nc.scalar.copy(xs_sb[:], ps_xs[:])
                    nc.sync.dma_start(cc_in[b_][:], xs_sb[:])

            # ================= AllReduce (per batch, overlaps phase A) ==========
            for b_ in range(B):
                nc.gpsimd.collective_compute(
                    "AllReduce", OP.add,
                    ins=[cc_in[b_].opt()], outs=[cc_out[b_].opt()],
                    replica_groups=[list(range(NCORES))])
                nc.sync.dma_start(dtin_sb[b_][:],
                                  cc_out[b_][0:DS, :].bitcast(F32R))

            # ---- dt phase: md = -softplus(dt_raw + b_dt) = ln(sigmoid(-(dt_raw + b_dt)))
            LTD = 512
            with tc.tile_pool(name="pd_ps", bufs=4, space="PSUM") as pdps:
                for b_ in range(B):
                    for ch in range(L // LTD):
                        t0 = ch * LTD
                        for blk in range(NBLK):
                            ps_dt = pdps.tile([128, LTD], F32, tag="ps_dt")
                            nc.tensor.matmul(
                                ps_dt[:], wdt_sb[:, blk * 128:(blk + 1) * 128],
                                dtin_sb[b_][:, t0:t0 + LTD],
                                start=True, stop=True)
                            nc.scalar.activation(
                                md[b_][blk][:, t0:t0 + LTD], ps_dt[:],
                                AF.Sigmoid, bias=bdt_sb[:, blk, :], scale=-1.0)
                for b_ in range(B):
                    for blk in range(NBLK):
                        nc.scalar.activation(md[b_][blk][:], md[b_][blk][:], AF.Ln)

            # ================= Phase B =================
            with (
                tc.tile_pool(name="pb", bufs=2) as pb,
                tc.tile_pool(name="pb_ps", bufs=4, space="PSUM") as pbps,
            ):
                NPT = L // 512
                for b_ in range(B):
                    yins = {}
                    for blk in range(NBLK):
                        # dtx = (-dt) * x_conv (sign fixed via negated W_x B-cols)
                        dtx = pb.tile([128, L], F32, tag="dtx")
                        nc.vector.tensor_mul(dtx[:], md[b_][blk][:], xcv[b_][blk][:])
                        y_ps = [pbps.tile([128, 512], F32, tag="y_ps", bufs=4,
                                          name=f"yps{b_}{blk}{pt}")
                                for pt in range(NPT)]
                        for n in range(DS):
                            # B_ssm[n,:] replicated over partitions
                            bb = pb.tile([128, L], F32, tag="bbn", bufs=3, name=f"bb{b_}{blk}{n}")
                            nc.sync.dma_start(
                                bb[:],
                                cc_out[b_][DS + n:DS + n + 1, :].broadcast_to([128, L]))
                            # dA_n = exp(A[:, n] * md)
                            da = pb.tile([128, L], F32, tag="dan", bufs=3, name=f"da{b_}{blk}{n}")
                            nc.scalar.activation(da[:], md[b_][blk][:], AF.Exp,
                                                 scale=a_sb[:, blk, n:n + 1])
                            # u_n = dtx * B_n
                            u = pb.tile([128, L], F32, tag="un", bufs=3, name=f"u{b_}{blk}{n}")
                            eng = nc.vector if n >= 13 else nc.gpsimd
                            eng.tensor_mul(u[:], dtx[:], bb[:])
                            # full-length scan, no chaining
                            h = pb.tile([128, L], BF16, tag="hn", name=f"h{b_}{blk}{n}")
                            nc.vector.tensor_tensor_scan(h[:], da[:], u[:],
                                                         0.0, OP.mult, OP.add)
                            # y += h_n on the tensor engine (identity matmul)
                            for pt in range(NPT):
                                nc.tensor.matmul(
                                    y_ps[pt][:], identb_sb[:],
                                    h[:, pt * 512:(pt + 1) * 512],
                                    start=(n == 0), stop=False)
                        # y += x_conv * D via diag(D) matmul, then gate from PSUM
                        yin = pb.tile([128, L], F32R, tag=f"yin{blk}", bufs=1)
                        for pt in range(NPT):
                            nc.tensor.matmul(
                                y_ps[pt][:], diagd_sb[:, blk, :],
                                xcv[b_][blk][:, pt * 512:(pt + 1) * 512],
                                start=False, stop=True)
                            nc.vector.tensor_mul(
                                yin[:, pt * 512:(pt + 1) * 512], y_ps[pt][:],
                                zac[b_][blk][:, pt * 512:(pt + 1) * 512])
                        yins[blk] = yin
                    # out_proj: out[t, dm] += yin.T @ wout
                    for mt in range(L // 128):
                        for dmh in range(2):
                            ps_o = pbps.tile([128, 512], F32, tag="ps_o")
                            for blk in range(NBLK):
                                nc.tensor.matmul(
                                    ps_o[:],
                                    yins[blk][:, mt * 128:(mt + 1) * 128],
                                    wout_sb[:, blk, dmh * 512:(dmh + 1) * 512],
                                    start=(blk == 0), stop=(blk == NBLK - 1))
                            osb = pb.tile([128, 512], F32, tag="osb")
                            nc.scalar.copy(osb[:], ps_o[:])
                            nc.sync.dma_start(
                                out_d[b_, mt * 128:(mt + 1) * 128,
                                      dmh * 512:(dmh + 1) * 512],
                                osb[:])

    nc.compile()
    return nc


_NC_CACHE = {}


def _get_nc():
    key = (LTB, SCAN_DT, Z_DT)
    if key not in _NC_CACHE:
        _NC_CACHE[key] = build_nc()
    return _NC_CACHE[key]


def make_in_maps(x, W_in, conv_w, conv_b, W_x, W_dt, b_dt, A_log, D, W_out):
    x = np.asarray(x, np.float32)
    W_in = np.asarray(W_in, np.float32)
    conv_w = np.asarray(conv_w, np.float32)
    conv_b = np.asarray(conv_b, np.float32)
    W_x = np.asarray(W_x, np.float32)
    W_dt = np.asarray(W_dt, np.float32)
    b_dt = np.asarray(b_dt, np.float32)
    A_log = np.asarray(A_log, np.float32)
    D = np.asarray(D, np.float32)
    W_out = np.asarray(W_out, np.float32)

    xt = np.ascontiguousarray(x.transpose(0, 2, 1)).reshape(B, KBLK, 128, L)
    A = np.exp(A_log)  # positive |A|; md = -dt on device

    in_maps = []
    for c in range(NCORES):
        lo = c * DIL
        sl = slice(lo, lo + DIL)
        in_maps.append({
            "x_t": xt,
            "win": np.ascontiguousarray(
                np.concatenate([W_in[:, sl], W_in[:, DI + lo:DI + lo + DIL]],
                               axis=1)),
            "wout": np.ascontiguousarray(W_out[sl]),
            "wx": np.ascontiguousarray(
                np.concatenate([W_x[sl, :DS], -W_x[sl, DS:]], axis=1)
            ).astype(ml_dtypes.bfloat16),
            "wdt": np.ascontiguousarray(W_dt[:, sl]),
            "a": np.ascontiguousarray(A[sl]),
            "convw": np.ascontiguousarray(conv_w[sl]),
            "convb": np.ascontiguousarray(conv_b[sl, None]),
            "dvec": np.ascontiguousarray(D[sl, None]),
            "bdt": np.ascontiguousarray(-b_dt[sl, None]),
            "identb": np.eye(128, dtype=ml_dtypes.bfloat16),
            "diagd": np.stack([np.diag(D[lo + k * 128:lo + (k + 1) * 128])
                               for k in range(NBLK)]).reshape(DIL, 128)
                       .astype(ml_dtypes.bfloat16),
        })
    return in_maps


def kernel(**inputs):
    nc = _get_nc()
    in_maps = make_in_maps(**inputs)
    res = run_bass_kernel_spmd(nc, in_maps, list(range(NCORES)))
    out = np.zeros((B, L, DM), np.float32)
    for c in range(NCORES):
        out += res.results[c]["out_p"]
    return out


# revision 13
# speedup vs baseline: 3.3743x; 1.3259x over previous
"""Mamba block (MockMambaBlock) on 8 Trainium2 NeuronCores.

Sharding: tensor-parallel over d_inner (8 x 256 channels), both batches on
every core. The x_proj/dt_proj contraction over d_inner is completed with an
on-device AllReduce of the small (B, 32, L) partial; out_proj row-partials
are summed on the host (the gather step).

Layout on device: channels on partitions, tokens along the free dimension,
with the SSM state index n laid out n-major along the free dim so the
per-(d,n) scan needs no cross-partition work. The d_state-broadcasts are
done with step-0 (broadcast) access patterns + a partition-replicating DMA.
"""

import sys

sys.path.insert(0, "/opt/trn_rl_repo")

import numpy as np
import ml_dtypes

import concourse.bass as bass
import concourse.bacc as bacc
import concourse.mybir as mybir
import concourse.tile as tile
from concourse.bass_utils import run_bass_kernel_spmd

F32 = mybir.dt.float32
F32R = mybir.dt.float32r
BF16 = mybir.dt.bfloat16
AF = mybir.ActivationFunctionType
OP = mybir.AluOpType

B, L, DM, DI, DS, DC = 2, 2048, 1024, 2048, 16, 4
NCORES = 8
DIL = DI // NCORES          # 256 channels per core
NBLK = DIL // 128           # 2 partition blocks of channels
KBLK = DM // 128            # 8 contraction blocks for in_proj
LTA = 512                   # phase A token chunk
LTB = 128                   # phase B token chunk
SCAN_DT = F32               # dtype for u/h scan tensors
Z_DT = BF16                 # dtype for silu(z) resident
XC_DT = BF16                # dtype for x_conv resident


def build_nc(ltb=LTB, scan_dt=SCAN_DT, z_dt=Z_DT, xc_dt=XC_DT):
    nc = bacc.Bacc()

    x_t = nc.dram_tensor("x_t", [B, KBLK, 128, L], BF16, kind="ExternalInput")
    win_d = nc.dram_tensor("win", [DM, 2 * DIL], BF16, kind="ExternalInput")
    wout_d = nc.dram_tensor("wout", [DIL, DM], BF16, kind="ExternalInput")
    wx_d = nc.dram_tensor("wx", [DIL, 2 * DS], BF16, kind="ExternalInput")
    wdt_d = nc.dram_tensor("wdt", [DS, DIL], BF16, kind="ExternalInput")
    a_d = nc.dram_tensor("a", [DIL, DS], F32, kind="ExternalInput")
    convw_d = nc.dram_tensor("convw", [DIL, DC], F32, kind="ExternalInput")
    convb_d = nc.dram_tensor("convb", [DIL, 1], F32, kind="ExternalInput")
    dvec_d = nc.dram_tensor("dvec", [DIL, 1], F32, kind="ExternalInput")
    bdt_d = nc.dram_tensor("bdt", [DIL, 1], F32, kind="ExternalInput")
    identb_d = nc.dram_tensor("identb", [128, 128], BF16, kind="ExternalInput")
    diagd_d = nc.dram_tensor("diagd", [DIL, 128], BF16, kind="ExternalInput")
    out_d = nc.dram_tensor("out_p", [B, L, DM], F32, kind="ExternalOutput")

    ncha = L // LTA
    nchb = L // ltb
    NMT = ltb // 128

    with tile.TileContext(nc) as tc:
        with (
            tc.tile_pool(name="weights", bufs=1) as wp,
            tc.tile_pool(name="resident", bufs=1) as rp,
            tc.tile_pool(name="dram", bufs=1, space="DRAM") as dp,
        ):
            # ---- weights to SBUF ----
            win_sb = wp.tile([128, KBLK, 2 * DIL], BF16)
            nc.sync.dma_start(win_sb[:], win_d[:].rearrange("(k p) m -> p k m", p=128))
            wout_sb = wp.tile([128, NBLK, DM], BF16)
            nc.sync.dma_start(wout_sb[:], wout_d[:].rearrange("(k p) m -> p k m", p=128))
            wx_sb = wp.tile([128, NBLK, 2 * DS], BF16)
            nc.sync.dma_start(wx_sb[:], wx_d[:].rearrange("(k p) m -> p k m", p=128))
            wdt_sb = wp.tile([DS, DIL], BF16)
            nc.sync.dma_start(wdt_sb[:], wdt_d[:])
            a_sb = wp.tile([128, NBLK, DS], F32)
            nc.sync.dma_start(a_sb[:], a_d[:].rearrange("(k p) m -> p k m", p=128))
            convw_sb = wp.tile([128, NBLK, DC], F32)
            nc.sync.dma_start(convw_sb[:], convw_d[:].rearrange("(k p) m -> p k m", p=128))
            convb_sb = wp.tile([128, NBLK, 1], F32)
            nc.sync.dma_start(convb_sb[:], convb_d[:].rearrange("(k p) m -> p k m", p=128))
            dvec_sb = wp.tile([128, NBLK, 1], F32)
            nc.sync.dma_start(dvec_sb[:], dvec_d[:].rearrange("(k p) m -> p k m", p=128))
            bdt_sb = wp.tile([128, NBLK, 1], F32)
            nc.sync.dma_start(bdt_sb[:], bdt_d[:].rearrange("(k p) m -> p k m", p=128))
            identb_sb = wp.tile([128, 128], BF16)
            nc.sync.dma_start(identb_sb[:], identb_d[:])
            diagd_sb = wp.tile([128, NBLK, 128], BF16)
            nc.sync.dma_start(diagd_sb[:], diagd_d[:].rearrange("(k p) m -> p k m", p=128))

            # ---- resident activations ----
            xcv = [[rp.tile([128, L], xc_dt, name=f"xcv{b_}{k}", tag=f"xcv{b_}{k}")
                    for k in range(NBLK)] for b_ in range(B)]
            zac = [[rp.tile([128, L], z_dt, name=f"zac{b_}{k}", tag=f"zac{b_}{k}")
                    for k in range(NBLK)] for b_ in range(B)]
            dtin_sb = [rp.tile([DS, L], BF16, name=f"dtin{b_}", tag=f"dtin{b_}")
                       for b_ in range(B)]
            md = [[rp.tile([128, L], BF16, name=f"md{b_}{k}", tag=f"md{b_}{k}")
                   for k in range(NBLK)] for b_ in range(B)]

            cc_in = [dp.tile([2 * DS, L], BF16, name=f"cc_in{b_}") for b_ in range(B)]
            cc_out = [dp.tile([2 * DS, L], BF16, addr_space="Shared",
                              name=f"cc_out{b_}") for b_ in range(B)]

            # ================= Phase A =================
            with (
                tc.tile_pool(name="pa", bufs=2) as pa,
                tc.tile_pool(name="pa_ps", bufs=4, space="PSUM") as paps,
                tc.tile_pool(name="pa_ps1", bufs=1, space="PSUM") as paps1,
            ):
                xp_buf = [pa.tile([128, LTA + DC - 1], F32, name=f"xpb{k}",
                                  tag=f"xpb{k}", bufs=1) for k in range(NBLK)]
                for b_ in range(B):
                    ps_xs = paps1.tile([2 * DS, L], F32, tag="ps_xs")
                    for ch in range(ncha):
                        t0 = ch * LTA
                        xs_all = pa.tile([128, KBLK, LTA], BF16, tag="xs_all")
                        nc.sync.dma_start(
                            xs_all[:],
                            x_t[b_].transpose([1, 0, 2])[:, :, t0:t0 + LTA])
                        for m in range(2 * NBLK):
                            ps = paps.tile([128, LTA], F32, tag="ps_in")
                            for kb in range(KBLK):
                                nc.tensor.matmul(
                                    ps[:],
                                    win_sb[:, kb, m * 128:(m + 1) * 128],
                                    xs_all[:, kb, :],
                                    start=(kb == 0), stop=(kb == KBLK - 1))
                            if m < NBLK:  # x branch: conv + silu
                                blk = m
                                if ch == 0:
                                    nc.vector.memset(xp_buf[blk][:, 0:DC - 1], 0.0)
                                else:
                                    nc.vector.tensor_copy(
                                        xp_buf[blk][:, 0:DC - 1],
                                        xp_buf[blk][:, LTA:LTA + DC - 1])
                                nc.scalar.copy(xp_buf[blk][:, DC - 1:LTA + DC - 1], ps[:])
                                cacc = pa.tile([128, LTA], F32, tag="cacc")
                                nc.vector.tensor_scalar_mul(
                                    cacc[:], xp_buf[blk][:, 0:LTA],
                                    convw_sb[:, blk, 0:1])
                                for k in range(1, DC):
                                    nc.vector.scalar_tensor_tensor(
                                        cacc[:], xp_buf[blk][:, k:k + LTA],
                                        convw_sb[:, blk, k:k + 1], cacc[:],
                                        OP.mult, OP.add)
                                nc.scalar.activation(
                                    xcv[b_][blk][:, t0:t0 + LTA], cacc[:],
                                    AF.Silu, bias=convb_sb[:, blk, :])
                            else:  # z branch: silu
                                blk = m - NBLK
                                nc.scalar.activation(
                                    zac[b_][blk][:, t0:t0 + LTA], ps[:], AF.Silu)
                        # x_proj partial for this chunk
                        for kb in range(NBLK):
                            nc.tensor.matmul(
                                ps_xs[:, t0:t0 + LTA],
                                wx_sb[:, kb, :],
                                xcv[b_][kb][:, t0:t0 + LTA],
                                start=(kb == 0), stop=(kb == NBLK - 1))
                    xs_sb = pa.tile([2 * DS, L], BF16, tag="xs_sb")
                    nc.scalar.copy(xs_sb[:], ps_xs[:])
                    nc.sync.dma_start(cc_in[b_][:], xs_sb[:])

            # ================= AllReduce (per batch, overlaps phase A) ==========
            for b_ in range(B):
                nc.gpsimd.collective_compute(
                    "AllReduce", OP.add,
                    ins=[cc_in[b_].opt()], outs=[cc_out[b_].opt()],
                    replica_groups=[list(range(NCORES))])
                nc.sync.dma_start(dtin_sb[b_][:],
                                  cc_out[b_][0:DS, :])

            # ---- dt phase: md = -softplus(dt_raw + b_dt) = ln(sigmoid(-(dt_raw + b_dt)))
            LTD = 512
            with tc.tile_pool(name="pd_ps", bufs=4, space="PSUM") as pdps:
                for b_ in range(B):
                    for ch in range(L // LTD):
                        t0 = ch * LTD
                        for blk in range(NBLK):
                            ps_dt = pdps.tile([128, LTD], F32, tag="ps_dt")
                            nc.tensor.matmul(
                                ps_dt[:], wdt_sb[:, blk * 128:(blk + 1) * 128],
                                dtin_sb[b_][:, t0:t0 + LTD],
                                start=True, stop=True)
                            nc.scalar.activation(
                                md[b_][blk][:, t0:t0 + LTD], ps_dt[:],
                                AF.Sigmoid, bias=bdt_sb[:, blk, :], scale=-1.0)
                for b_ in range(B):
                    for blk in range(NBLK):
                        nc.scalar.activation(md[b_][blk][:], md[b_][blk][:], AF.Ln)

            # ================= Phase B =================
            with (
                tc.tile_pool(name="pb", bufs=2) as pb,
                tc.tile_pool(name="pb_ps", bufs=4, space="PSUM") as pbps,
            ):
                NPT = L // 512
                for b_ in range(B):
                    yins = {}
                    for blk in range(NBLK):
                        # dtx = (-dt) * x_conv (sign fixed via negated W_x B-cols)
                        dtx = pb.tile([128, L], BF16, tag="dtx")
                        nc.vector.tensor_mul(dtx[:], md[b_][blk][:], xcv[b_][blk][:])
                        y_ps = [pbps.tile([128, 512], F32, tag="y_ps", bufs=4,
                                          name=f"yps{b_}{blk}{pt}")
                                for pt in range(NPT)]
                        for n in range(DS):
                            # B_ssm[n,:] replicated over partitions
                            bb = pb.tile([128, L], BF16, tag="bbn", bufs=3, name=f"bb{b_}{blk}{n}")
                            nc.sync.dma_start(
                                bb[:],
                                cc_out[b_][DS + n:DS + n + 1, :].broadcast_to([128, L]))
                            # dA_n = exp(A[:, n] * md)
                            da = pb.tile([128, L], F32, tag="dan", bufs=3, name=f"da{b_}{blk}{n}")
                            nc.scalar.activation(da[:], md[b_][blk][:], AF.Exp,
                                                 scale=a_sb[:, blk, n:n + 1])
                            # u_n = dtx * B_n
                            u = pb.tile([128, L], BF16, tag="un", bufs=3, name=f"u{b_}{blk}{n}")
                            nc.vector.tensor_mul(u[:], dtx[:], bb[:])
                            # full-length scan, no chaining
                            h = pb.tile([128, L], BF16, tag="hn", name=f"h{b_}{blk}{n}")
                            nc.vector.tensor_tensor_scan(h[:], da[:], u[:],
                                                         0.0, OP.mult, OP.add)
                            # y += h_n on the tensor engine (identity matmul)
                            for pt in range(NPT):
                                nc.tensor.matmul(
                                    y_ps[pt][:], identb_sb[:],
                                    h[:, pt * 512:(pt + 1) * 512],
                                    start=(n == 0), stop=False)
                        # y += x_conv * D via diag(D) matmul, then gate from PSUM
                        yin = pb.tile([128, L], BF16, tag=f"yin{blk}", bufs=1)
                        for pt in range(NPT):
                            nc.tensor.matmul(
                                y_ps[pt][:], diagd_sb[:, blk, :],
                                xcv[b_][blk][:, pt * 512:(pt + 1) * 512],
                                start=False, stop=True)
                            nc.vector.tensor_mul(
                                yin[:, pt * 512:(pt + 1) * 512], y_ps[pt][:],
                                zac[b_][blk][:, pt * 512:(pt + 1) * 512])
                        yins[blk] = yin
                    # out_proj: out[t, dm] += yin.T @ wout
                    for mt in range(L // 128):
                        for dmh in range(2):
                            ps_o = pbps.tile([128, 512], F32, tag="ps_o")
                            for blk in range(NBLK):
                                nc.tensor.matmul(
                                    ps_o[:],
                                    yins[blk][:, mt * 128:(mt + 1) * 128],
                                    wout_sb[:, blk, dmh * 512:(dmh + 1) * 512],
                                    start=(blk == 0), stop=(blk == NBLK - 1))
                            osb = pb.tile([128, 512], F32, tag="osb")
                            nc.scalar.copy(osb[:], ps_o[:])
                            nc.sync.dma_start(
                                out_d[b_, mt * 128:(mt + 1) * 128,
                                      dmh * 512:(dmh + 1) * 512],
                                osb[:])

    nc.compile()
    return nc


_NC_CACHE = {}


def _get_nc():
    key = (LTB, SCAN_DT, Z_DT)
    if key not in _NC_CACHE:
        _NC_CACHE[key] = build_nc()
    return _NC_CACHE[key]


def make_in_maps(x, W_in, conv_w, conv_b, W_x, W_dt, b_dt, A_log, D, W_out):
    x = np.asarray(x, np.float32)
    W_in = np.asarray(W_in, np.float32)
    conv_w = np.asarray(conv_w, np.float32)
    conv_b = np.asarray(conv_b, np.float32)
    W_x = np.asarray(W_x, np.float32)
    W_dt = np.asarray(W_dt, np.float32)
    b_dt = np.asarray(b_dt, np.float32)
    A_log = np.asarray(A_log, np.float32)
    D = np.asarray(D, np.float32)
    W_out = np.asarray(W_out, np.float32)

    xt = np.ascontiguousarray(x.transpose(0, 2, 1)).reshape(B, KBLK, 128, L).astype(ml_dtypes.bfloat16)
    A = np.exp(A_log)  # positive |A|; md = -dt on device

    in_maps = []
    for c in range(NCORES):
        lo = c * DIL
        sl = slice(lo, lo + DIL)
        in_maps.append({
            "x_t": xt,
            "win": np.ascontiguousarray(
                np.concatenate([W_in[:, sl], W_in[:, DI + lo:DI + lo + DIL]],
                               axis=1)).astype(ml_dtypes.bfloat16),
            "wout": np.ascontiguousarray(W_out[sl]).astype(ml_dtypes.bfloat16),
            "wx": np.ascontiguousarray(
                np.concatenate([W_x[sl, :DS], -W_x[sl, DS:]], axis=1)
            ).astype(ml_dtypes.bfloat16),
            "wdt": np.ascontiguousarray(W_dt[:, sl]).astype(ml_dtypes.bfloat16),
            "a": np.ascontiguousarray(A[sl]),
            "convw": np.ascontiguousarray(conv_w[sl]),
            "convb": np.ascontiguousarray(conv_b[sl, None]),
            "dvec": np.ascontiguousarray(D[sl, None]),
            "bdt": np.ascontiguousarray(-b_dt[sl, None]),
            "identb": np.eye(128, dtype=ml_dtypes.bfloat16),
            "diagd": np.stack([np.diag(D[lo + k * 128:lo + (k + 1) * 128])
                               for k in range(NBLK)]).reshape(DIL, 128)
                       .astype(ml_dtypes.bfloat16),
        })
    return in_maps


def kernel(**inputs):
    nc = _get_nc()
    in_maps = make_in_maps(**inputs)
    res = run_bass_kernel_spmd(nc, in_maps, list(range(NCORES)))
    out = np.zeros((B, L, DM), np.float32)
    for c in range(NCORES):
        out += res.results[c]["out_p"]
    return out
